# revision 21
# baseline (speedup 1.0000x reference)
"""LIF spike (vanilla) Trainium2 kernel — time-packed spikes, split gate.

Reference recurrence over leading time dim T (per element):
    u_t = TAU * u_{t-1} * (1 - o_{t-1}) + x_t
    o_t = (u_t - VTH > 0) ? 1.0 : 0.0

Chunks of [128, F] are processed in PAIRS with interleaved emission so
the two dependency chains fill each other's engine stalls.  Per step:

    S1 (DVE):  u[:dv] = (c mult 0.5) add x ; u[dv:] = (d mult 0.25) add x
    S2 (ACT):  s = Sign(u - VTH) in {-1,+1} bf16
    S3 (PE):   psum_j += 2^t * s_j          scaled-identity matmuls
    S4 gate (t<T-1):
        cols [0:dv)  (DVE):    c = (u is_le VTH) mult u
        cols [dv:F)  (gpsimd): r = u*s ; d = u - r   (= 2*u*(1-o))

All fp32 arithmetic matches the jax fp32 reference bit-for-bit: s=+-1
products and pow-2 scales are exact, so 0.25*d == 0.5*(u*(1-o)) and the
final add rounds identically to the reference's fused update.

After the 8 steps of a chunk, ACT decodes the pack psum:
    byte = 0.5*psum + 127.5 in {0..255} (bit t == o_t), stored uint8.
Host expands bits -> f32.  Only inexact corner: u == VTH exactly
(Sign=0 corrupts one byte; ~1e-8/element odds).

Sharding: pure data parallel over batch dim B=64 -> 8 cores x 8 batches.
Per core HBM traffic: 32 MiB in (f32) + 1 MiB out (uint8)."""

import numpy as np

T = 8
B = 64
C = 128
H = 32
W = 32
NCORES = 8
BS = B // NCORES            # batches per core
N = BS * C * H * W          # 1,048,576 elements per time step per core
P = 128                     # SBUF partitions
F = 2048                    # tile free-dim
NCHUNK = N // (P * F)       # spatial chunks per core
TAU = 0.5
VTH = 0.99999
MMF = 512                   # matmul moving free dim (= one PSUM bank of f32)

PACK_DTYPE = "uint8"        # packed byte dtype in DRAM
GP_COLS = 640               # gate columns on gpsimd (s-product trick)


def _build(nt=T, nchunk=NCHUNK, fdim=F, pack_dtype=PACK_DTYPE,
           gp_cols=GP_COLS):
    import concourse.bacc as bacc
    import concourse.mybir as mybir
    import concourse.tile as tile

    f32 = mybir.dt.float32
    bf16 = mybir.dt.bfloat16
    pdt = getattr(mybir.dt, pack_dtype)
    alu = mybir.AluOpType
    act = mybir.ActivationFunctionType
    nbank = fdim // MMF
    dv = fdim - gp_cols
    nc = bacc.Bacc("TRN2", target_bir_lowering=False)
    x = nc.dram_tensor("x", [nt, nchunk, P, fdim], f32, kind="ExternalInput")
    w = nc.dram_tensor("w", [P, nt, P], bf16, kind="ExternalInput")
    o = nc.dram_tensor("o", [nchunk, P, fdim], pdt, kind="ExternalOutput")
    assert nchunk % 2 == 0

    with tile.TileContext(nc) as tc:
        with (
            tc.tile_pool(name="wp", bufs=1) as wp,
            tc.tile_pool(name="xp", bufs=6) as xp,
            tc.tile_pool(name="op", bufs=4) as opl,
            tc.tile_pool(name="up", bufs=4) as up,
            tc.tile_pool(name="cp", bufs=4) as cp,
            tc.tile_pool(name="dp", bufs=4) as dp,
            tc.tile_pool(name="rp", bufs=4) as rp,
            tc.tile_pool(name="pk", bufs=2) as pk,
            tc.tile_pool(name="ps", bufs=2, space="PSUM") as ps,
        ):
            wt = wp.tile([P, nt, P], bf16)
            nc.sync.dma_start(wt[:], w[:])
            nvth = wp.tile([P, 1], f32)
            nc.vector.memset(nvth[:], -VTH)
            dbias = wp.tile([P, 1], f32)
            nc.vector.memset(dbias[:], 127.5)

            for pair in range(nchunk // 2):
                chunks = [2 * pair, 2 * pair + 1]
                pst, u, ct, dt = {}, {}, {}, {}
                for ci in chunks:
                    pst[ci] = [ps.tile([P, MMF], f32, name=f"ps{j}",
                                       tag=f"ps{j}") for j in range(nbank)]
                    u[ci] = xp.tile([P, fdim], f32, name="u0", tag="xt")
                    nc.sync.dma_start(u[ci][:], x[0, ci])
                for t in range(nt):
                    for ci in chunks:
                        if t > 0:
                            xt = xp.tile([P, fdim], f32, name="xt", tag="xt")
                            nc.sync.dma_start(xt[:], x[t, ci])
                            un = up.tile([P, fdim], f32)
                            nc.vector.scalar_tensor_tensor(
                                un[:, :dv], ct[ci][:], TAU, xt[:, :dv],
                                alu.mult, alu.add)
                            if gp_cols:
                                nc.vector.scalar_tensor_tensor(
                                    un[:, dv:], dt[ci][:], 0.25, xt[:, dv:],
                                    alu.mult, alu.add)
                            u[ci] = un
                        st = opl.tile([P, fdim], bf16)
                        nc.scalar.activation(st[:], u[ci][:], act.Sign,
                                             bias=nvth[:], scale=1.0)
                        for j in range(nbank):
                            nc.tensor.matmul(
                                pst[ci][j][:], wt[:, t],
                                st[:, j * MMF:(j + 1) * MMF],
                                start=(t == 0), stop=(t == nt - 1))
                        if t < nt - 1:
                            if gp_cols:
                                rt = rp.tile([P, gp_cols], f32)
                                nc.gpsimd.tensor_tensor(
                                    rt[:], u[ci][:, dv:], st[:, dv:],
                                    alu.mult)
                                dn = dp.tile([P, gp_cols], f32)
                                nc.gpsimd.tensor_tensor(
                                    dn[:], u[ci][:, dv:], rt[:],
                                    alu.subtract)
                                dt[ci] = dn
                            cn = cp.tile([P, dv], f32)
                            nc.vector.scalar_tensor_tensor(
                                cn[:], u[ci][:, :dv], VTH, u[ci][:, :dv],
                                alu.is_le, alu.mult)
                            ct[ci] = cn
                for ci in chunks:
                    pkt = pk.tile([P, fdim], pdt)
                    for j in range(nbank):
                        nc.scalar.activation(
                            pkt[:, j * MMF:(j + 1) * MMF], pst[ci][j][:],
                            act.Identity, bias=dbias[:], scale=0.5)
                    nc.sync.dma_start(o[ci], pkt[:])
    nc.finalize()
    return nc


def _weights():
    # lhsT layout [K=P, t, M=P]: w[k, t, m] = 2^t * (k == m)
    w = np.zeros((P, T, P), np.float32)
    for t in range(T):
        np.fill_diagonal(w[:, t, :], float(2 ** t))
    import ml_dtypes

    return w.astype(ml_dtypes.bfloat16)


def _decode(o):
    """Packed byte values -> f32 spike train [T, ...]."""
    o = np.asarray(o)
    if o.dtype == np.uint8:
        b = o
    elif o.dtype == np.uint16:  # bf16 bits
        b = np.rint((o.astype(np.uint32) << 16).view(np.float32)).astype(np.uint8)
    else:
        b = np.rint(np.asarray(o, dtype=np.float32)).astype(np.uint8)
    out = np.empty((T,) + b.shape, np.float32)
    for t in range(T):
        out[t] = ((b >> t) & 1).astype(np.float32)
    return out


def make_in_maps(x):
    wb = _weights()
    in_maps = []
    for i in range(NCORES):
        s = np.ascontiguousarray(x[:, i * BS: (i + 1) * BS])
        in_maps.append({"x": s.reshape(T, NCHUNK, P, F), "w": wb})
    return in_maps


def kernel(x):
    x = np.ascontiguousarray(np.asarray(x, dtype=np.float32))
    assert x.shape == (T, B, C, H, W), x.shape
    from concourse.bass_utils import run_bass_kernel_spmd

    nc = _build()
    res = run_bass_kernel_spmd(nc, make_in_maps(x), core_ids=list(range(NCORES)))
    out = np.empty((T, B, C, H, W), np.float32)
    for i, r in enumerate(res.results):
        out[:, i * BS: (i + 1) * BS] = _decode(r["o"]).reshape(T, BS, C, H, W)
    return out


# revision 25
# speedup vs baseline: 1.3223x; 1.3223x over previous
"""LIF spike (vanilla) Trainium2 kernel — time-packed spikes.

Reference recurrence over leading time dim T (per element):
    u_t = TAU * u_{t-1} * (1 - o_{t-1}) + x_t
    o_t = (u_t - VTH > 0) ? 1.0 : 0.0

Per time step (u carried in SBUF, x streamed from HBM):
    S1 (DVE):  u = (c mult TAU) add x_t       scalar_tensor_tensor, t>0
    S2 (ACT):  s = Sign(u - VTH) in {-1,+1}   bf16 out
    S3 (PE):   psum_j += 2^t * s_j            scaled-identity matmuls
    S4 (DVE):  c = (u is_le VTH) mult u       scalar_tensor_tensor, t<T-1

All fp32 arithmetic matches the jax fp32 reference bit-for-bit (mult by
TAU=0.5 and {0,1} masks exact; Sign sees the same fp32 u - VTH).

After the 8 steps of a chunk, ACT decodes the pack psum:
    byte = 0.5*psum + 127.5 in {0..255} (bit t == o_t), stored uint8.
Host expands bits -> f32.  Only inexact corner: u == VTH exactly
(Sign=0 corrupts one byte; ~1e-8/element odds).

Startup: the first chunk's x[0]/x[1] loads and first gate/update are
issued in interleaved quarter-slices so DVE starts ~1.5us in.  Tail:
the last chunk's final step runs sign/pack/decode/store in halves.

Sharding: pure data parallel over batch dim B=64 -> 8 cores x 8 batches.
Per core HBM traffic: 32 MiB in (f32) + 1 MiB out (uint8)."""

import numpy as np

T = 8
B = 64
C = 128
H = 32
W = 32
NCORES = 8
BS = B // NCORES            # batches per core
N = BS * C * H * W          # 1,048,576 elements per time step per core
P = 128                     # SBUF partitions
F = 4096                    # tile free-dim
NCHUNK = N // (P * F)       # spatial chunks per core
TAU = 0.5
VTH = 0.99999
MMF = 512                   # matmul moving free dim (= one PSUM bank of f32)

PACK_DTYPE = "uint8"        # packed byte dtype in DRAM


def _build(nt=T, nchunk=NCHUNK, fdim=F, xb=3, ob=2, ub=2, cb=2,
           pack_dtype=PACK_DTYPE):
    import concourse.bacc as bacc
    import concourse.mybir as mybir
    import concourse.tile as tile

    f32 = mybir.dt.float32
    bf16 = mybir.dt.bfloat16
    pdt = getattr(mybir.dt, pack_dtype)
    alu = mybir.AluOpType
    act = mybir.ActivationFunctionType
    nbank = fdim // MMF
    Q = fdim // 4
    nc = bacc.Bacc("TRN2", target_bir_lowering=False)
    x = nc.dram_tensor("x", [nt, nchunk, P, fdim], f32, kind="ExternalInput")
    w = nc.dram_tensor("w", [P, nt, P], bf16, kind="ExternalInput")
    o = nc.dram_tensor("o", [nchunk, P, fdim], pdt, kind="ExternalOutput")

    with tile.TileContext(nc) as tc:
        with (
            tc.tile_pool(name="wp", bufs=1) as wp,
            tc.tile_pool(name="xp", bufs=xb) as xp,
            tc.tile_pool(name="op", bufs=ob) as opl,
            tc.tile_pool(name="up", bufs=ub) as up,
            tc.tile_pool(name="cp", bufs=cb) as cp,
            tc.tile_pool(name="pk", bufs=2) as pk,
            tc.tile_pool(name="ps", bufs=1, space="PSUM") as ps,
        ):
            wt = wp.tile([P, nt, P], bf16)
            nc.sync.dma_start(wt[:], w[:])
            nvth = wp.tile([P, 1], f32)
            nc.vector.memset(nvth[:], -VTH)
            dbias = wp.tile([P, 1], f32)
            nc.vector.memset(dbias[:], 127.5)

            def gate(ct, u, sliced=False):
                # c = (u is_le VTH) mult u
                if sliced:  # quarter slices (startup only)
                    for q in range(4):
                        sl = slice(q * Q, (q + 1) * Q)
                        nc.vector.scalar_tensor_tensor(
                            ct[:, sl], u[:, sl], VTH, u[:, sl],
                            alu.is_le, alu.mult)
                else:
                    nc.vector.scalar_tensor_tensor(
                        ct[:], u[:], VTH, u[:], alu.is_le, alu.mult)

            for i in range(nchunk):
                first = i == 0
                last = i == nchunk - 1
                pst = [ps.tile([P, MMF], f32, name=f"ps{j}", tag=f"ps{j}")
                       for j in range(nbank)]
                u0 = xp.tile([P, fdim], f32, name="u0", tag="xt")
                x1 = None
                if first:
                    # interleaved quarter loads of x[0] and x[1] so the
                    # first gate/update slices start ~1.5us in
                    x1 = xp.tile([P, fdim], f32, name="x1", tag="xt")
                    order = [(u0, 0, 0), (x1, 1, 0), (u0, 0, 1), (x1, 1, 1),
                             (u0, 0, 2), (x1, 1, 2), (u0, 0, 3), (x1, 1, 3)]
                    for tgt, tt_, q in order:
                        sl = slice(q * Q, (q + 1) * Q)
                        nc.sync.dma_start(tgt[:, sl], x[tt_, i][:, sl])
                else:
                    nc.sync.dma_start(u0[:], x[0, i])
                u = u0
                for t in range(nt):
                    if t > 0:
                        if first and t == 1:
                            xt = x1
                        else:
                            xt = xp.tile([P, fdim], f32, name="xt", tag="xt")
                            nc.sync.dma_start(xt[:], x[t, i])
                        un = up.tile([P, fdim], f32)
                        if (first and t == 1) or (last and t == nt - 1):
                            for q in range(4):
                                sl = slice(q * Q, (q + 1) * Q)
                                nc.vector.scalar_tensor_tensor(
                                    un[:, sl], ct[:, sl], TAU, xt[:, sl],
                                    alu.mult, alu.add)
                        else:
                            nc.vector.scalar_tensor_tensor(
                                un[:], ct[:], TAU, xt[:], alu.mult, alu.add)
                        u = un
                    if last and t == nt - 1:
                        # fully sliced tail: update above was skipped for
                        # the last step; do upd/sign/pack/decode/store in
                        # quarter-column slices so the drain pipelines
                        pkt = pk.tile([P, fdim], pdt)
                        st = opl.tile([P, fdim], bf16)
                        for q in range(4):
                            qs = slice(q * Q, (q + 1) * Q)
                            nc.scalar.activation(st[:, qs], u[:, qs],
                                                 act.Sign, bias=nvth[:],
                                                 scale=1.0)
                            for j in range(q * nbank // 4,
                                           (q + 1) * nbank // 4):
                                nc.tensor.matmul(
                                    pst[j][:], wt[:, t],
                                    st[:, j * MMF:(j + 1) * MMF],
                                    start=(t == 0), stop=True)
                                nc.scalar.activation(
                                    pkt[:, j * MMF:(j + 1) * MMF],
                                    pst[j][:], act.Identity,
                                    bias=dbias[:], scale=0.5)
                            nc.sync.dma_start(o[i][:, qs], pkt[:, qs])
                        continue
                    st = opl.tile([P, fdim], bf16)
                    nc.scalar.activation(st[:], u[:], act.Sign,
                                         bias=nvth[:], scale=1.0)
                    for j in range(nbank):
                        nc.tensor.matmul(
                            pst[j][:], wt[:, t],
                            st[:, j * MMF:(j + 1) * MMF],
                            start=(t == 0), stop=(t == nt - 1))
                    if t < nt - 1:
                        ct = cp.tile([P, fdim], f32)
                        gate(ct, u, sliced=(first and t == 0))
                if not last:
                    pkt = pk.tile([P, fdim], pdt)
                    for j in range(nbank):
                        nc.scalar.activation(
                            pkt[:, j * MMF:(j + 1) * MMF], pst[j][:],
                            act.Identity, bias=dbias[:], scale=0.5)
                    nc.sync.dma_start(o[i], pkt[:])
    nc.finalize()
    return nc


def _weights():
    # lhsT layout [K=P, t, M=P]: w[k, t, m] = 2^t * (k == m)
    w = np.zeros((P, T, P), np.float32)
    for t in range(T):
        np.fill_diagonal(w[:, t, :], float(2 ** t))
    import ml_dtypes

    return w.astype(ml_dtypes.bfloat16)


def _decode(o):
    """Packed byte values -> f32 spike train [T, ...]."""
    o = np.asarray(o)
    if o.dtype == np.uint8:
        b = o
    elif o.dtype == np.uint16:  # bf16 bits
        b = np.rint((o.astype(np.uint32) << 16).view(np.float32)).astype(np.uint8)
    else:
        b = np.rint(np.asarray(o, dtype=np.float32)).astype(np.uint8)
    out = np.empty((T,) + b.shape, np.float32)
    for t in range(T):
        out[t] = ((b >> t) & 1).astype(np.float32)
    return out


def make_in_maps(x):
    wb = _weights()
    in_maps = []
    for i in range(NCORES):
        s = np.ascontiguousarray(x[:, i * BS: (i + 1) * BS])
        in_maps.append({"x": s.reshape(T, NCHUNK, P, F), "w": wb})
    return in_maps


def kernel(x):
    x = np.ascontiguousarray(np.asarray(x, dtype=np.float32))
    assert x.shape == (T, B, C, H, W), x.shape
    from concourse.bass_utils import run_bass_kernel_spmd

    nc = _build()
    res = run_bass_kernel_spmd(nc, make_in_maps(x), core_ids=list(range(NCORES)))
    out = np.empty((T, B, C, H, W), np.float32)
    for i, r in enumerate(res.results):
        out[:, i * BS: (i + 1) * BS] = _decode(r["o"]).reshape(T, BS, C, H, W)
    return out
